# revision 1
# baseline (speedup 1.0000x reference)
"""Trainium2 Bass kernel for nn_CorrelationModule (B=4, C=64, H=W=64).

Per batch b (q = query[b].reshape(C,N), ex = exemplar[b].reshape(C,N), N=4096):
  ex_corr = (W_lin @ ex)^T                      # [N, C]
  A       = ex_corr @ q                         # [N, N]
  Sm      = softmax(A, axis=-1)                 # row softmax
  att     = q @ Sm^T                            # [C, N]
  out     = leaky_relu(BN(conv3x3(att)), 0.1)

Sharding: 8 cores = (batch b, image-half h). Each core computes att for its
2048 output pixels plus one 64-pixel halo row on each side (S=2176 pixel
slice), then convolves locally. No collectives.

Per-core pipeline (i = pixel in slice, j = key pixel, d/c = channels).
Columns are processed in 5 phases (512,512,512,384,256); keys in slots of
3 j-tiles (3x128 keys). Per slot: 3 K=64 fp32r matmuls (row-group packed by
tile parity) produce A'[j,i] into a [128,1536] 3-bank PSUM slot; ONE wide
ScalarE exp (PSUM->SBUF, no max subtraction needed: |A| < 60 fits fp32);
3 K=128 matmuls accumulate [1|qT]^T @ P' into the phase accumulator
(row 0 = softmax denominator since qTa column 0 is ones). Phase tail:
reciprocal of row 0 -> mask -> gpsimd broadcast -> normalized write into a
zero-padded [65,34,66] fp16 conv buffer whose partition 64 is ones (carries
the BN bias via weight row 64; BN scale folded into weights host-side).
Conv chunks (9 accumulated fp16 matmuls each) are interleaved into later
phases as their pbuf rows become available; leaky relu on DVE; store.

Schedule notes (TimelineSim-driven; modeled 87.5us vs 101.4us for the
previous version):
- ScalarE exp is the bottleneck: 69.5us busy (58us streaming floor at
  1 elem/cycle/lane @1.2GHz + 222-cycle per-op access overhead). The
  [128,1536] slots (vs [128,1024] pairs) cut the op count 80 -> 55.
- A' matmuls are issued TWO slots ahead via a global slot list: A'(k+2)
  and att(k) share the same trigger (exp(k) frees the A-slot buffer), so
  the PE stream never makes ACT wait at slot or phase boundaries.
- PSUM: A-slot pool 2x6KB + att/pc/excT pool 2x2KB = 16KB exactly.
- Startup: W2/exs/q arrive in small first chunks on two DGE queues
  (sync + gpsimd); pbuf memsets go to gpsimd so the DVE queue serves the
  excT copies immediately; first exp ~5us.
- Tail: last conv chunk splits into a tail3-dependent and a small
  tail4-dependent block (PSUM from the A pool, whose slots are free by
  then) so only ~256 output columns drain after the last exp.
Remaining headroom if iterating further: per-core halo recompute (5.9%
of exp work) could move to an inter-core exchange; the last-phase
broadcast could use a PE ones-matmul instead of gpsimd; PE-side A'/att
could run the halo phase in bf16.
"""

import numpy as np

B, C, H, W = 4, 64, 64, 64
N = H * W
HALF = N // 2            # 2048 output pixels per core
S = HALF + 2 * W         # 2176 slice incl. halo rows
NJT = N // 128           # 32 j-tiles
PH = [(0, 512), (512, 512), (1024, 512), (1536, 384), (1920, 256)]
SLOT = 3                 # j-tiles per exp slot
NSLOT = 11               # 10 full slots + 1 two-tile slot per phase
EPS = 1e-5
N_CORES = 8

_cache = {}


def _build(n_iters=0):
    """Build+compile the SPMD module. n_iters>0 wraps the body in a HW loop
    (benchmark mode)."""
    import concourse.bacc as bacc
    import concourse.tile as tile
    from concourse import mybir
    from concourse.bass import ts

    F32 = mybir.dt.float32
    R = mybir.dt.float32r
    F16 = mybir.dt.float16
    Exp = mybir.ActivationFunctionType.Exp
    Lrelu = mybir.ActivationFunctionType.Lrelu
    MUL = mybir.AluOpType.mult
    MAX = mybir.AluOpType.max

    nc = bacc.Bacc("TRN2", target_bir_lowering=False, debug=False,
                   num_devices=N_CORES)
    exs_d = nc.dram_tensor("exs", [C, S], R, kind="ExternalInput").ap()
    q_d = nc.dram_tensor("q128", [128, N], R, kind="ExternalInput").ap()
    qTa_d = nc.dram_tensor("qTa", [128, NJT * 65], R, kind="ExternalInput").ap()
    W2_d = nc.dram_tensor("W2", [C, 128], R, kind="ExternalInput").ap()
    w9_d = nc.dram_tensor("w9", [65, 9 * 64], F16, kind="ExternalInput").ap()
    mask_d = nc.dram_tensor("mask", [1, S], F32, kind="ExternalInput").ap()
    y_d = nc.dram_tensor("yout", [C, HALF], F32, kind="ExternalOutput").ap()

    def body(cp, Pp, Ap, ap, sp):
        # ---- input loads, chunked so compute starts early ----
        exs_t = cp.tile([C, S], R, name="exs_t", tag="exs_t", bufs=2)
        W2_t = cp.tile([C, 128], R, name="W2_t", tag="W2_t")
        q_t = cp.tile([128, N], R, name="q_t", tag="q_t", bufs=2)
        qTa_t = cp.tile([128, NJT, 65], R, name="qTa_t", tag="qTa_t", bufs=2)
        w9_t = cp.tile([65, 9, 64], F16, name="w9_t", tag="w9_t")
        mask_t = cp.tile([1, S], F32, name="mask_t", tag="mask_t")

        # DMAs fan out over two DGE queues (sync + gpsimd; the scalar queue
        # would pollute the ACT sequencer). Small first chunks (W2, exs 512,
        # q 512) so the first exp starts as early as possible — the shared
        # DMA transfer resource serves them in issue order.
        nc.sync.dma_start(out=exs_t[:, 0:512], in_=exs_d[:, 0:512])
        nc.sync.dma_start(out=W2_t, in_=W2_d)
        nc.gpsimd.dma_start(out=q_t[:, 0:512], in_=q_d[:, 0:512])
        nc.sync.dma_start(out=qTa_t[:, 0:8, :],
                          in_=qTa_d[:, 0:8 * 65]
                          .rearrange("p (a b) -> p a b", b=65))
        nc.gpsimd.dma_start(out=q_t[:, 512:1024], in_=q_d[:, 512:1024])
        nc.sync.dma_start(out=exs_t[:, 512:S], in_=exs_d[:, 512:S])
        for k in range(1, 4):
            nc.gpsimd.dma_start(out=q_t[:, ts(k, 1024)], in_=q_d[:, ts(k, 1024)])
            nc.sync.dma_start(out=qTa_t[:, ts(k, 8), :],
                              in_=qTa_d[:, ts(k, 8 * 65)]
                              .rearrange("p (a b) -> p a b", b=65))
        nc.sync.dma_start(out=w9_t, in_=w9_d.rearrange("p (a b) -> p a b", b=64))
        nc.sync.dma_start(out=mask_t, in_=mask_d)

        # ---- conv buffer: partition 64 is ones (BN bias row) ----
        rs64 = sp.tile([65, S], F32, name="rs64", tag="rs64")
        rs0 = sp.tile([1, S], F32, name="rs0", tag="rs0")
        rb = sp.tile([64, S], F32, name="rb", tag="rb")
        pbuf = sp.tile([65, 34, W + 2], F16, name="pbuf", tag="pbuf")

        # ---- excT = (W_lin @ ex) duplicated onto both partition halves ----
        # (PSUM from the att pool so the A-slot rotation never waits on it;
        # pbuf memsets AFTER the copies so the DVE queue serves them first)
        excT_t = cp.tile([128, S], R, name="excT_t", tag="excT_t", bufs=2)
        for c0, cw in ((0, 512), (512, 512), (1024, 512), (1536, 512),
                       (2048, 128)):
            pe = ap.tile([128, 512], F32, name="pe_mm0", tag="att")
            nc.tensor.matmul(pe[:, 0:cw], W2_t, exs_t[:, c0:c0 + cw],
                             start=True, stop=True)
            nc.vector.tensor_copy(out=excT_t[:, c0:c0 + cw], in_=pe[:, 0:cw])
        # memsets on gpsimd: the DVE queue must stay clear for the excT
        # copies on the first-exp critical path (pbuf isn't read until ~20us)
        nc.gpsimd.memset(pbuf[0:64, :, :], 0.0)
        nc.gpsimd.memset(pbuf[64:65, :, :], 1.0)

        # ---- conv3x3 (+BN bias via ones row) + leaky relu + store ----
        def conv_chunk(t4, pc=None, blocks=((0, 8),), use_act=False):
            """Conv over output-row blocks [rb0, rb0+nr) of chunk t4 (each
            block is one PSUM accumulation group at pc cols rb0*64)."""
            if pc is None:
                pc = ap.tile([128, 512], F32, name=f"pc_conv{t4}", tag="att")
            for rb0, nr in blocks:
                r0 = 1 + 8 * t4 + rb0
                c0 = 64 * rb0
                for tap in range(9):
                    dy, dx = tap // 3, tap % 3
                    nc.tensor.matmul(
                        pc[0:64, c0:c0 + 64 * nr], w9_t[:, tap, :],
                        pbuf[:, r0 - 1 + dy:r0 - 1 + nr + dy, dx:dx + W],
                        start=(tap == 0), stop=(tap == 8),
                    )
                # leaky relu + store per block so early blocks drain early.
                # The final chunk uses ScalarE Lrelu (ACT is idle by then,
                # DVE is busy with the tail chain); mid-kernel chunks keep
                # DVE so ACT never loses exp throughput.
                cw = 64 * nr
                yo = sp.tile([64, 512], F32, name=f"yo_{t4}", tag="yo",
                             bufs=2)
                if use_act:
                    nc.scalar.activation(yo[:, 0:cw], pc[0:64, c0:c0 + cw],
                                         Lrelu, alpha=0.1)
                else:
                    y1 = sp.tile([64, 512], F32, name=f"y1_{t4}", tag="y1",
                                 bufs=2)
                    nc.vector.tensor_scalar_mul(y1[:, 0:cw],
                                                pc[0:64, c0:c0 + cw], 0.1)
                    nc.vector.tensor_tensor(out=yo[:, 0:cw],
                                            in0=pc[0:64, c0:c0 + cw],
                                            in1=y1[:, 0:cw], op=MAX)
                nc.sync.dma_start(out=y_d[:, 512 * t4 + c0:512 * t4 + c0 + cw],
                                  in_=yo[:, 0:cw])

        # ---- attention phases ----
        def slot_mms(pi, si):
            """A' matmuls for slot si of phase pi (None-safe)."""
            if pi >= len(PH):
                return
            p0, pw = PH[pi]
            sz = SLOT if si < NSLOT - 1 else NJT - SLOT * (NSLOT - 1)
            A_sl = Ap.tile([128, 1536], F32, name=f"A_{pi}_{si}", tag="A")
            for tl in range(sz):
                t = SLOT * si + tl
                h0 = (t % 2) * 64
                nc.tensor.matmul(A_sl[:, 512 * tl:512 * tl + pw],
                                 q_t[h0:h0 + 64, ts(t, 128)],
                                 excT_t[h0:h0 + 64, p0:p0 + pw],
                                 start=True, stop=True)
            return A_sl

        # Global slot sequence with A' issued TWO slots ahead: A'(k+2) and
        # att(k) share the same trigger (exp(k) done via the 2-deep A-slot
        # rotation), so the PE stream never makes ACT wait at slot or phase
        # boundaries.
        slots = [(pi, si) for pi in range(len(PH)) for si in range(NSLOT)]
        A_q = [slot_mms(0, 0), slot_mms(0, 1)]
        att_p = None
        for k, (pi, si) in enumerate(slots):
            p0, pw = PH[pi]
            if si == 0:
                att_f = ap.tile([128, 512], F32, name=f"att{pi}", tag="att")
                att_p = att_f[0:65, :]
            if k + 2 < len(slots):
                A_q.append(slot_mms(*slots[k + 2]))
            sz = SLOT if si < NSLOT - 1 else NJT - SLOT * (NSLOT - 1)
            P_sl = Pp.tile([128, SLOT, 512], R, name=f"P_{pi}_{si}", tag="P")
            nc.scalar.activation(
                P_sl[:, 0:sz, 0:pw],
                A_q[k].rearrange("p (g x) -> p g x", g=SLOT)[:, 0:sz, 0:pw],
                Exp)
            for tl in range(sz):
                t = SLOT * si + tl
                nc.tensor.matmul(att_p[:, 0:pw], qTa_t[:, t, :],
                                 P_sl[:, tl, 0:pw],
                                 start=(t == 0), stop=(t == NJT - 1))
            if si != NSLOT - 1:
                continue

            # phase tail: denominator reciprocal (row 64; partition starts
            # must be multiples of 32, so the denom lives at 64 and hops to
            # partition 0 via a small SBUF DMA) -> mask -> broadcast ->
            # normalized write into the padded conv buffer
            sl = slice(p0, p0 + pw)
            nc.vector.reciprocal(out=rs64[64:65, sl], in_=att_p[64:65, 0:pw])
            nc.sync.dma_start(out=rs0[:, sl], in_=rs64[64:65, sl])
            if pi == 0 or pi == len(PH) - 1:
                nc.vector.tensor_tensor(out=rs0[:, sl], in0=rs0[:, sl],
                                        in1=mask_t[:, sl], op=MUL)
            nc.gpsimd.partition_broadcast(rb[:, sl], rs0[0:1, sl])
            r0, nr = p0 // W, pw // W
            nc.vector.tensor_tensor(
                out=pbuf[0:64, r0:r0 + nr, 1:W + 1],
                in0=att_p[0:64, 0:pw].rearrange("p (r c) -> p r c", c=W),
                in1=rb[:, sl].rearrange("p (r c) -> p r c", c=W),
                op=MUL,
            )
            # conv chunk pi-2: its pbuf rows were completed by phase pi-1's
            # tail, so its matmuls never block the PE stream here
            if pi >= 2:
                conv_chunk(pi - 2)

        # last chunk: rows 24-27 only need tail3 (overlap with tail4); rows
        # 28-31 need tail4. PSUM from the A pool, whose slots are long free.
        pcA = Ap.tile([128, 1536], F32, name="pc_conv3", tag="A")
        conv_chunk(3, pc=pcA, blocks=((0, 4), (4, 4)))

    with tile.TileContext(nc) as tc:
        with tc.tile_pool(name="cp", bufs=1) as cp, \
             tc.tile_pool(name="Pp", bufs=4) as Pp, \
             tc.tile_pool(name="Ap", bufs=2, space="PSUM") as Ap, \
             tc.tile_pool(name="ap", bufs=2, space="PSUM") as ap, \
             tc.tile_pool(name="sp", bufs=1) as sp:
            if n_iters > 0:
                with tc.For_i(0, n_iters, 1):
                    body(cp, Pp, Ap, ap, sp)
            else:
                body(cp, Pp, Ap, ap, sp)

    nc.compile()
    return nc


def _prep_in_maps(exemplar, query, W_lin, conv_w, gamma, beta, run_mean,
                  run_var):
    exemplar = np.asarray(exemplar, dtype=np.float32)
    query = np.asarray(query, dtype=np.float32)
    W_lin = np.asarray(W_lin, dtype=np.float32)
    conv_w = np.asarray(conv_w, dtype=np.float32)
    gamma = np.asarray(gamma, dtype=np.float32)
    beta = np.asarray(beta, dtype=np.float32)
    run_mean = np.asarray(run_mean, dtype=np.float32)
    run_var = np.asarray(run_var, dtype=np.float32)

    s = gamma / np.sqrt(run_var + EPS)               # [C]
    t = beta - run_mean * s                          # [C]
    # w9[i, 3*dy+dx, o] = conv_w[o, i, dy, dx] * s[o]; row 64 of center tap = t
    w9 = np.zeros((65, 9, 64), dtype=np.float32)
    ws = conv_w * s[:, None, None, None]             # [o, i, 3, 3]
    w9[0:64] = ws.transpose(1, 2, 3, 0).reshape(64, 9, 64)
    w9[64, 4, :] = t
    w9 = w9.reshape(65, 9 * 64).astype(np.float16)

    W2 = np.concatenate([W_lin.T, W_lin.T], axis=1)  # [C, 128]

    in_maps = []
    for core in range(N_CORES):
        b, h = core // 2, core % 2
        ex_flat = exemplar[b].reshape(C, N)
        q_flat = query[b].reshape(C, N)
        i_lo = h * HALF - W
        s0, s1 = max(0, i_lo), min(N, i_lo + S)
        exs = np.zeros((C, S), dtype=np.float32)
        exs[:, s0 - i_lo:s1 - i_lo] = ex_flat[:, s0:s1]
        mask = np.zeros((1, S), dtype=np.float32)
        mask[0, s0 - i_lo:s1 - i_lo] = 1.0
        # qTa column 64 is ones (softmax denominator lands in att row 64,
        # a legal 32-aligned partition start for the reciprocal read)
        qTa = np.empty((128, NJT, 65), dtype=np.float32)
        qTa[:, :, 0:64] = q_flat.T.reshape(NJT, 128, 64).transpose(1, 0, 2)
        qTa[:, :, 64] = 1.0
        in_maps.append({
            "exs": exs,
            "q128": np.ascontiguousarray(np.concatenate([q_flat, q_flat],
                                                        axis=0)),
            "qTa": np.ascontiguousarray(qTa.reshape(128, NJT * 65)),
            "W2": np.ascontiguousarray(W2),
            "w9": np.ascontiguousarray(w9),
            "mask": mask,
        })
    return in_maps


def _run(in_maps, n_iters=0):
    from concourse import bass_utils
    key = ("nc", n_iters)
    if key not in _cache:
        _cache[key] = _build(n_iters)
    nc = _cache[key]
    return bass_utils.run_bass_kernel_spmd(nc, in_maps,
                                           core_ids=list(range(N_CORES)))


def kernel(exemplar, query, W_lin, conv_w, gamma, beta, run_mean, run_var):
    in_maps = _prep_in_maps(exemplar, query, W_lin, conv_w, gamma, beta,
                            run_mean, run_var)
    res = _run(in_maps)
    out = np.empty((B, C, H, W), dtype=np.float32)
    for core in range(N_CORES):
        b, h = core // 2, core % 2
        out[b, :, h * 32:(h + 1) * 32, :] = \
            res.results[core]["yout"].reshape(C, 32, W)
    return out

